# revision 6
# baseline (speedup 1.0000x reference)
"""BiLIF (bidirectional leaky-integrate-and-fire) node on 8 Trainium2 NeuronCores.

Problem: inputs [T=16, B=64, N=65536] f32.
  s1 = LIF-scan(x,          tau=4/3, v_th=0.75)   (hard reset to 0)
  s2 = LIF-scan(flip(x, 0), tau=4/3, v_th=1.25)
  out = (s1 + s2) / 2

Strategy (v2)
  - Shard the batch dim across the 8 cores (pure data parallel). Per core:
    8*65536 positions = 128 partitions x 4096 columns, two [128, 2048]
    column chunks. Both direction scans run concurrently: at step t the
    forward scan consumes x[t], the backward scan consumes x[15-t], so
    out[t] completes at step t and every x tile is loaded exactly once.
  - DVE does ONLY the two fused LIF step passes (charge+reset as one
    2-src custom op; 2-src fp32 custom DVE is 1x = 0.96 G elem/lane/s,
    the hard floor). t=0 uses single-src tensor_scalar at 2x.
  - ACT produces the spikes: sigma = Sign(h - th) -> bf16, both dirs.
  - PE combines AND packs: 4 matmuls per chunk-step with base-3 pack
    weights. W_A[2q, q] = 0.5, W_A[2q+1, q] = 1.5 (cols 64.. zero);
    W_B same pattern shifted to cols 64..127. psum[q, f] then holds
    p = t0 + 3*t1 with t = (sig1+sig2)/2 in {-1,0,1} for partition pair
    (2q, 2q+1): rows 0:64 pack sigma cols 0:1024 (strip A), rows 64:128
    pack cols 1024:2048 (strip B). Zero weight columns contribute zeros,
    so all four matmuls accumulate into one [128, 1024] psum tile.
  - ACT copies psum [128, 1024] f32 -> fp8e4m3 (p in {-4..4} step 0.5 is
    exact in e4m3), so output DMA is 0.5 B/elem: 4.2 MB/core instead of
    8.4 MB. DMA per core: 33.6 MB in + 4.2 MB out ~= 114 us at the
    ~330 GB/s/core effective DMA roofline. The copy is emitted one step
    late so the in-order ACT queue never stalls waiting for PE.
  - Host decodes: t1 = round(p/3), t0 = p - 3*t1, out = (t+1)/2.
  Engine model per rep/core: DVE ~128 us, ACT ~128 us, PE ~57 us,
  DMA ~114 us -- a balanced ridge vs the 139 us all-DVE baseline.
"""

import numpy as np
import ml_dtypes  # noqa: F401  (fp8 dtype availability)

import concourse.bacc as bacc
import concourse.mybir as mybir
import concourse.tile as tile
import concourse.dve_ops as dve_ops
from concourse.dve_ops import DveOp
from concourse.dve_spec import (
    C0,
    C1,
    Spec,
    Src0,
    Src1,
    Zero,
    _has_src1,
    lower,
    select,
)
from concourse.dve_uop import DveOpSpec
from concourse import bass_utils

T, B, N = 16, 64, 65536
NCORES = 8
BS = B // NCORES        # batch rows per core
POS = BS * N            # independent positions per core
P = 128
FREE = POS // P         # 4096 columns per partition
CHUNK = 2048
NCHUNK = FREE // CHUNK
HALF = CHUNK // 2       # 1024: packed output columns per chunk
R = 0.75                # fl32(1 / fl32(4/3)) == 0.75 exactly
TH1, TH2 = 0.75, 1.25
F32 = mybir.dt.float32
BF16 = mybir.dt.bfloat16
FP8 = mybir.dt.float8e4
AF = mybir.ActivationFunctionType


def _register(name: str, spec: Spec) -> DveOp:
    """Register a custom DVE op at runtime (uops sha computed here)."""
    if name in dve_ops._SUB_OPCODE_FOR_NAME:
        for op in dve_ops.OPS:
            if op.name == name:
                return op
    row = dve_ops._CUSTOM_DVE_ROW_BASE + len(dve_ops.OPS)
    assert row < 0x20, "custom DVE opcode rows exhausted"
    sha = {}
    for ver in ("v3", "v4"):
        s = DveOpSpec(name=name, opcode=row, uops=lower(spec, ver=ver),
                      rd1_en=_has_src1(spec))
        sha[ver] = s.sha(ver)
    op = DveOp(name, spec, subdim=False, uops_sha=sha)
    dve_ops.OPS.append(op)
    dve_ops._SUB_OPCODE_FOR_NAME[name] = row
    dve_ops.CUSTOM_DVE_SPECS[name] = spec
    return op


_vp_node = select(Src1 < C1, Src1, Zero)
BILIF_STEP = _register(
    "BILIF_STEP",
    Spec(
        body=(Src0 - _vp_node) * C0 + _vp_node,
        reference=lambda in0, in1, s0, s1, imm2: (
            (in0 - np.where(in1 < s1, in1, 0).astype(np.float32))
            * np.float32(s0)
            + np.where(in1 < s1, in1, 0).astype(np.float32)
        ),
    ),
)


def _pack_weights() -> np.ndarray:
    """[128, 256] bf16: W_A = [:, :128], W_B = [:, 128:]. Both map
    partition pair (2q, 2q+1) -> row q with weights (0.5, 1.5); W_A
    writes psum rows 0:64, W_B rows 64:128; all other columns zero."""
    w = np.zeros((128, 256), np.float32)
    for q in range(64):
        w[2 * q, q] = 0.5
        w[2 * q + 1, q] = 1.5
        w[2 * q, 128 + 64 + q] = 0.5
        w[2 * q + 1, 128 + 64 + q] = 1.5
    return w.astype(ml_dtypes.bfloat16)


_NC_CACHE = {}


def _build_nc(repeat: int = 1):
    """Build + compile the SPMD per-core program. `repeat` replays the body
    (used only for steady-state timing experiments)."""
    key = repeat
    if key in _NC_CACHE:
        return _NC_CACHE[key]
    nc = bacc.Bacc("TRN2", target_bir_lowering=False, debug=False,
                   num_devices=NCORES)
    x_d = nc.dram_tensor("x", [T * P, FREE], F32, kind="ExternalInput").ap()
    w_d = nc.dram_tensor("w", [P, 2 * P], BF16, kind="ExternalInput").ap()
    o_d = nc.dram_tensor("o", [T * P, FREE // 2], FP8,
                         kind="ExternalOutput").ap()

    with tile.TileContext(nc) as tc:
        with tc.tile_pool(name="xp", bufs=16) as xp, \
             tc.tile_pool(name="h1p", bufs=3) as h1p, \
             tc.tile_pool(name="h2p", bufs=3) as h2p, \
             tc.tile_pool(name="a1p", bufs=3) as a1p, \
             tc.tile_pool(name="a2p", bufs=3) as a2p, \
             tc.tile_pool(name="outp", bufs=4) as outp, \
             tc.tile_pool(name="psp", bufs=2, space="PSUM") as psp, \
             tc.tile_pool(name="zp", bufs=1) as zp:
            wa = zp.tile([P, P], BF16, tag="wa", name="wa")
            nc.sync.dma_start(out=wa[:], in_=w_d[:, :P])
            wb = zp.tile([P, P], BF16, tag="wb", name="wb")
            nc.sync.dma_start(out=wb[:], in_=w_d[:, P:])
            b1 = zp.tile([P, 1], F32, tag="b1", name="b1")
            nc.vector.memset(b1[:], -TH1)
            b2 = zp.tile([P, 1], F32, tag="b2", name="b2")
            nc.vector.memset(b2[:], -TH2)
            for rep in range(repeat):
                for k in range(NCHUNK):
                    c0 = k * CHUNK
                    # Load each x[t] tile once, in first-use order
                    # (fwd uses t at step t, bwd uses t at step 15-t).
                    xt = {}
                    for t in [v for s in range(T // 2) for v in (s, T - 1 - s)]:
                        xt[t] = xp.tile([P, CHUNK], F32, tag="x",
                                        name=f"x{rep}_{k}_{t}")
                        nc.sync.dma_start(
                            out=xt[t][:],
                            in_=x_d[t * P:(t + 1) * P, c0:c0 + CHUNK])
                    h1_prev, h2_prev = None, None
                    pending = None  # (ps, t) awaiting copy+store
                    for t in range(T):
                        h1 = h1p.tile([P, CHUNK], F32, tag="h1", name="h1")
                        h2 = h2p.tile([P, CHUNK], F32, tag="h2", name="h2")
                        if t == 0:
                            # v = 0: h = 0.75*x exactly; single-src fp32
                            # tensor_scalar streams at 2x (2-port mode)
                            nc.vector.tensor_scalar(
                                out=h1[:], in0=xt[0][:], scalar1=R,
                                scalar2=None, op0=mybir.AluOpType.mult)
                            nc.vector.tensor_scalar(
                                out=h2[:], in0=xt[T - 1][:], scalar1=R,
                                scalar2=None, op0=mybir.AluOpType.mult)
                        else:
                            nc.vector._custom_dve(BILIF_STEP, out=h1[:],
                                                  in0=xt[t][:],
                                                  in1=h1_prev[:],
                                                  s0=R, s1=TH1)
                            nc.vector._custom_dve(BILIF_STEP, out=h2[:],
                                                  in0=xt[T - 1 - t][:],
                                                  in1=h2_prev[:],
                                                  s0=R, s1=TH2)
                        # sigma = sign(h - th) in bf16, exact on {-1,0,1}
                        a1 = a1p.tile([P, CHUNK], BF16, tag="a1", name="a1")
                        nc.scalar.activation(out=a1[:], in_=h1[:],
                                             func=AF.Sign, bias=b1[:],
                                             scale=1.0)
                        a2 = a2p.tile([P, CHUNK], BF16, tag="a2", name="a2")
                        nc.scalar.activation(out=a2[:], in_=h2[:],
                                             func=AF.Sign, bias=b2[:],
                                             scale=1.0)
                        # Drain the previous step's psum while PE works on
                        # this step (keeps the in-order ACT queue flowing).
                        if pending is not None:
                            _drain(nc, outp, o_d, pending, c0)
                        # Pack-combine: 4 accumulating matmuls.
                        ps = psp.tile([P, HALF], F32, tag="ps", name="ps")
                        for j in (0, 512):  # one PSUM bank (512 f32) each
                            sl = slice(j, j + 512)
                            sh = slice(HALF + j, HALF + j + 512)
                            nc.tensor.matmul(ps[:, sl], wa[:], a1[:, sl],
                                             start=True, stop=False)
                            nc.tensor.matmul(ps[:, sl], wa[:], a2[:, sl],
                                             start=False, stop=False)
                            nc.tensor.matmul(ps[:, sl], wb[:], a1[:, sh],
                                             start=False, stop=False)
                            nc.tensor.matmul(ps[:, sl], wb[:], a2[:, sh],
                                             start=False, stop=True)
                        pending = (ps, t)
                        h1_prev, h2_prev = h1, h2
                    _drain(nc, outp, o_d, pending, c0)

    nc.compile()
    _NC_CACHE[key] = nc
    return nc


def _drain(nc, outp, o_d, pending, c0):
    """ACT copy psum -> fp8 (p in {-4..4} step .5 is exact), then store."""
    ps, t = pending
    o = outp.tile([P, HALF], FP8, tag="o", name="o")
    nc.scalar.activation(out=o[:], in_=ps[:], func=AF.Copy,
                         bias=0.0, scale=1.0)
    nc.sync.dma_start(
        out=o_d[t * P:(t + 1) * P, c0 // 2:c0 // 2 + HALF], in_=o[:])


def _run(inputs: np.ndarray, repeat: int = 1, **kwargs):
    nc = _build_nc(repeat)
    w = _pack_weights()
    in_maps = []
    for c in range(NCORES):
        shard = np.ascontiguousarray(
            inputs[:, c * BS:(c + 1) * BS, :]).reshape(T * P, FREE)
        in_maps.append({"x": shard, "w": w})
    return bass_utils.run_bass_kernel_spmd(
        nc, in_maps, core_ids=list(range(NCORES)), **kwargs)


def _decode(o8: np.ndarray) -> np.ndarray:
    """[T*P, FREE//2] fp8 packed base-3 -> [T, BS, N] f32 output.

    Packed tile row q (resp. 64+q) col f of chunk k holds
    p = t0 + 3*t1 for partitions (2q, 2q+1) at column k*2048 + f
    (resp. + 1024 + f), with t = (sig1+sig2)/2; out = (t+1)/2."""
    p = o8.astype(np.float32).reshape(T, P, NCHUNK, HALF)
    t1 = np.round(p / 3.0)
    t0 = p - 3.0 * t1
    out = np.empty((T, P, FREE), np.float32)
    for k in range(NCHUNK):
        for band, cols in ((0, slice(k * CHUNK, k * CHUNK + HALF)),
                           (64, slice(k * CHUNK + HALF, (k + 1) * CHUNK))):
            out[:, 0:P:2, cols] = (t0[:, band:band + 64, k, :] + 1.0) * 0.5
            out[:, 1:P:2, cols] = (t1[:, band:band + 64, k, :] + 1.0) * 0.5
    return out.reshape(T, BS, N)


def kernel(inputs: np.ndarray, **kwargs) -> np.ndarray:
    inputs = np.asarray(inputs)
    assert inputs.shape == (T, B, N) and inputs.dtype == np.float32
    res = None
    err = None
    for _attempt in range(3):  # retry transient device faults
        try:
            res = _run(inputs, **kwargs)
            break
        except Exception as e:  # noqa: BLE001
            err = e
    if res is None:
        raise err
    out = np.empty((T, B, N), np.float32)
    for c in range(NCORES):
        out[:, c * BS:(c + 1) * BS, :] = _decode(res.results[c]["o"])
    return out
